# revision 5
# baseline (speedup 1.0000x reference)
"""Trainium2 Bass kernel for nn_Attention_83330955478086 (v10, bf16).

Full attention layer: QKV projections + (degenerate) rotary + causal softmax
attention + output projection.  x:(1,2048,4096), 32 heads x 128 head_dim.

Sharding: tensor-parallel over heads. Each of the 8 cores computes 4 heads
(d-shard of 512) of Q/K/V, runs attention for those heads, AllGathers the
attention outputs per 512-seq chunk (bf16, pipelined against remaining
attention work) and computes a 512-column slice of the final output
projection.  Host concatenates the slices.

v4 changes vs v2/v3:
  - matmul emission order groups runs of 8 consecutive matmuls into the
    SAME psum bank (profiling shows ~50ns/matmul extra when consecutive
    matmuls target different banks: same-bank runs hit 214ns for a
    512-col bf16 matmul, bank-rotating streams 260-264ns).
  - phase A: 1024-seq pair chunks with all 32 x k-tiles resident (x read
    from HBM once); wv streamed during the V pass (frees SBUF).
  - causal masking as a resident-bf16 0/1 staircase multiply on the exp
    output on the Vector engine (GpSimd tensor ops measured ~10x slower
    -> v3's gpsimd version serialized phase B).  Removes all mask DMA.
    Non-causal masks fall back to the v2 psum add path.
  - PE + exp-table warmup block at t=0 (dummy matmuls while the first
    weight/x DMAs land, so the HAM clock gate is released before the
    real work arrives).
  - phase B is software-pipelined: group g+1's score matmuls are emitted
    BEFORE group g's PV/k-sum matmuls, so the in-order PE never sits
    waiting for the exp of the group it just scored.

Layout: everything on-chip is "transposed" ([feature, seq]):
  - host feeds x^T, wq^T, wk^T, wv^T, wo^T (bf16)
  - Q/K projections emit Q^T/K^T tiles [head_dim, seq] into SBUF
  - scores are computed transposed: scoresT[k,q] = sum_hd K^T[hd,k]*Q^T[hd,q]
  - softmax: exp on ACT (1/sqrt(128) folded into the activation scale);
    k-sums via ones-matmul; bcast-reciprocal normalize
  - PV uses V in natural [seq, d] layout as stationary and emits attn^T
    [hd, q]; AllGather concatenates attn^T on the feature axis
  - output projection emits out^T [512, 2048] fp32; host transposes back.

The rotary pair-swap in the reference is the identity, so rotary is an
elementwise scale; wq/wk columns are permuted per head on the host (even hd
first, odd hd second) so the device multiplies by a [cos-sin; cos+sin]
stacked tile; the permutation cancels in the q.k contraction.
"""
import math
import os

import ml_dtypes
import numpy as np

import concourse.bacc as bacc
import concourse.tile as tile
from concourse.tile import add_dep_helper
from concourse import mybir
from concourse.bass_utils import run_bass_kernel_spmd

N_CORES = 8
S = 2048
D = 4096
H = 32
HD = 128
DSH = D // N_CORES  # 512 per-core d shard
HL = DSH // HD  # 4 heads per core
KT = D // 128  # 32 contraction tiles for the projections
SC = S // 512  # 4 seq chunks of 512
ST = S // 128  # 16 seq tiles of 128

F32 = mybir.dt.float32
BF16 = mybir.dt.bfloat16

# mask-block classes (per [128k, 512q] tile)
B_SKIP = 0  # fully masked (mask < -1e4): exp underflows to exactly 0 -> skip
B_ZERO = 1  # mask identically 0: skip the add
B_ADD = 2  # mixed: stream the mask tile and add


def _w_load(nc, sb_tile, dram, engine, chunks):
    """Load a [D, DSH] weight into sb_tile (kc-major [128, KT*DSH]).
    `chunks` is a list of kc counts (summing to KT); emitted in order."""
    kc0 = 0
    for n in chunks:
        getattr(nc, engine).dma_start(
            sb_tile[:, kc0 * DSH : (kc0 + n) * DSH].rearrange(
                "p (t d) -> p t d", d=DSH
            ),
            dram.ap()[kc0 * 128 : (kc0 + n) * 128, :].rearrange(
                "(t p) d -> p t d", p=128
            ),
        )
        kc0 += n


def _build(cls_grid, causal):
    nc = bacc.Bacc(
        "TRN2", target_bir_lowering=False, debug=False, num_devices=N_CORES
    )

    xT = nc.dram_tensor("xT", [D, S], BF16, kind="ExternalInput")
    wqT = nc.dram_tensor("wqT", [D, DSH], BF16, kind="ExternalInput")
    wkT = nc.dram_tensor("wkT", [D, DSH], BF16, kind="ExternalInput")
    wvT = nc.dram_tensor("wvT", [D, DSH], BF16, kind="ExternalInput")
    woT = nc.dram_tensor("woT", [D, DSH], BF16, kind="ExternalInput")
    gk_d = nc.dram_tensor("gk_d", [128, S], F32, kind="ExternalInput")
    warm_d = nc.dram_tensor("warm_d", [128, 512], BF16, kind="ExternalInput")
    ones_in = nc.dram_tensor("ones_in", [128, 1], BF16, kind="ExternalInput")
    if causal:
        stair_d = nc.dram_tensor("stair_d", [128, 2048], BF16, kind="ExternalInput")
    else:
        maskT = nc.dram_tensor("maskT", [S, S], F32, kind="ExternalInput")
    outT = nc.dram_tensor("outT", [DSH, S], F32, kind="ExternalOutput")

    attn_sc = [nc.dram_tensor(f"attn_sc{i}", [DSH, 512], BF16) for i in range(SC)]
    attn_full = [
        nc.dram_tensor(f"attn_full{i}", [D, 512], BF16, addr_space="Shared")
        for i in range(SC)
    ]

    inv_sqrt_hd = 1.0 / math.sqrt(HD)

    with tile.TileContext(nc) as tc, tc.tile_pool(
        name="persist", bufs=1
    ) as persist:
        qT_sb = persist.tile([128, HL * S], BF16, name="qT_sb")
        kT_sb = persist.tile([128, HL * S], BF16, name="kT_sb")
        v_sb = persist.tile([128, ST * DSH], BF16, name="v_sb")
        gk_sb = persist.tile([128, S], F32, name="gk_sb")
        ones_sb = persist.tile([128, 1], BF16, name="ones_sb")
        if causal:
            stair_sb = persist.tile([128, 2048], BF16, name="stair_sb")
            nc.scalar.dma_start(stair_sb[:], stair_d.ap())

        nc.scalar.dma_start(ones_sb[:], ones_in.ap())
        nc.scalar.dma_start(gk_sb[:], gk_d.ap())

        # ---------------- phase A: Q/K/V projections ----------------
        # Two 1024-seq pair chunks; per pair the full x^T [4096, 1024] is
        # resident so x is read once and Q/K stationary loads serve two
        # 512-wide matmuls each.
        with (
            tc.tile_pool(name="pw", bufs=1) as pw,
            tc.tile_pool(name="px", bufs=1) as px,
            tc.tile_pool(name="pwv", bufs=2) as pwv,
            tc.tile_pool(name="pa_ps", bufs=1, space="PSUM") as pa_ps,
        ):
            wq_sb = pw.tile([128, KT * DSH], BF16, name="wq_sb")
            wk_sb = pw.tile([128, KT * DSH], BF16, name="wk_sb")
            # warmup: release the PE HAM clock gate + load the exp table
            # while the first weight/x DMAs land; uses the main pools' first
            # ring slots (a separate pool scope costs ~10us of teardown).
            scr = px.tile([128, 512], BF16, name="scr")
            nc.sync.dma_start(scr[:], warm_d.ap())
            scr2 = px.tile([128, 16], BF16, name="scr2")
            wps = pa_ps.tile([128, 512], F32, name="pp0")
            for i in range(14):
                nc.tensor.matmul(
                    wps[:], scr[:, 0:128], scr[:], start=(i == 0), stop=(i == 13)
                )
            nc.scalar.activation(
                scr2[:], scr[:, 0:16], mybir.ActivationFunctionType.Exp
            )
            _w_load(nc, wq_sb, wqT, "gpsimd", [1, 3, 4, 8, 16])
            _w_load(nc, wk_sb, wkT, "gpsimd", [2, 6, 8, 16])

            for pr in range(2):  # 1024-seq pair chunks
                p0 = pr * 1024
                xts = []
                for kc in range(KT):
                    xt = px.tile([128, 1024], BF16, name=f"xt{kc}")
                    eng = nc.scalar if (pr == 0 and kc % 2 == 1) else nc.sync
                    eng.dma_start(
                        xt[:], xT.ap()[kc * 128 : (kc + 1) * 128, p0 : p0 + 1024]
                    )
                    xts.append(xt)

                def qk_pass(w_sb, out_sb):
                    ps = [
                        pa_ps.tile([128, 512], F32, name=f"pp{i}")
                        for i in range(8)
                    ]
                    for blk in range(KT // 8):
                        for b in range(8):  # bank = dt*2 + half
                            dt, half = b // 2, b % 2
                            for kc in range(blk * 8, blk * 8 + 8):
                                nc.tensor.matmul(
                                    ps[b][:],
                                    w_sb[
                                        :,
                                        kc * DSH
                                        + dt * 128 : kc * DSH
                                        + (dt + 1) * 128,
                                    ],
                                    xts[kc][:, half * 512 : (half + 1) * 512],
                                    start=(kc == 0),
                                    stop=(kc == KT - 1),
                                )
                    for dt in range(HL):
                        for half in range(2):
                            s0 = p0 + half * 512
                            nc.vector.tensor_mul(
                                out_sb[:, dt * S + s0 : dt * S + s0 + 512],
                                ps[dt * 2 + half][:],
                                gk_sb[:, s0 : s0 + 512],
                            )

                qk_pass(wq_sb, qT_sb)
                qk_pass(wk_sb, kT_sb)

                # V pass: x stationary (resident), wv streamed as moving
                psv = [
                    pa_ps.tile([128, 512], F32, name=f"pp{i}") for i in range(8)
                ]
                for blk in range(KT // 8):
                    wvt = []
                    for kc in range(blk * 8, blk * 8 + 8):
                        wv_t = pwv.tile([128, 512], BF16, name=f"wv_t{kc % 8}")
                        nc.scalar.dma_start(
                            wv_t[:], wvT.ap()[kc * 128 : (kc + 1) * 128, :]
                        )
                        wvt.append(wv_t)
                    for st in range(8):
                        for kc in range(blk * 8, blk * 8 + 8):
                            nc.tensor.matmul(
                                psv[st][:],
                                xts[kc][:, st * 128 : (st + 1) * 128],
                                wvt[kc % 8][:],
                                start=(kc == 0),
                                stop=(kc == KT - 1),
                            )
                for st in range(8):
                    gt = pr * 8 + st  # global 128-seq tile
                    nc.vector.tensor_copy(
                        v_sb[:, gt * DSH : (gt + 1) * DSH], psv[st][:]
                    )

        # ------ phase B+C: attention, AllGather, output projection ------
        with (
            tc.tile_pool(name="pwo", bufs=1) as pwo,
            tc.tile_pool(name="p2_m", bufs=2) as p2_m,
            tc.tile_pool(name="p2_ex", bufs=4) as p2_ex,
            tc.tile_pool(name="p2_sm", bufs=2) as p2_sm,
            tc.tile_pool(name="p2_at", bufs=3) as p2_at,
            tc.tile_pool(name="p3_a", bufs=4) as p3_a,
            tc.tile_pool(name="p3_ev", bufs=4) as p3_ev,
        ):
            wo_sb = pwo.tile([128, KT * DSH], BF16, name="wo_sb")
            _w_load(nc, wo_sb, woT, "gpsimd", [8, 8, 8, 8])

            last_b = {}
            pend_ag = []
            with (
                tc.tile_pool(name="p2_sc", bufs=2, space="PSUM") as pS,
                tc.tile_pool(name="p2_ap", bufs=2, space="PSUM") as pAtt,
                tc.tile_pool(name="p2_sp", bufs=2, space="PSUM") as pSum,
            ):
                for qc in range(SC):
                    q0 = qc * 512
                    live = [kt for kt in range(ST) if cls_grid[kt][qc] != B_SKIP]
                    groups = [live[i : i + 2] for i in range(0, len(live), 2)]
                    for h in range(HL):
                        att_ps = pAtt.tile([128, 512], F32, name="att_ps")
                        sum_ps = pSum.tile([1, 512], F32, name="sum_ps")
                        n_mm = sum(len(g) for g in groups)
                        mm = 0

                        def flush(pend):
                            # PV + k-sum matmuls for a completed group; the
                            # PE reaches these only after the NEXT group's
                            # score matmuls, hiding the exp latency.
                            nonlocal mm
                            group, ex = pend
                            for i, kt in enumerate(group):
                                nc.tensor.matmul(
                                    att_ps[:],
                                    v_sb[:, kt * DSH + h * 128 : kt * DSH + (h + 1) * 128],
                                    ex[:, i * 512 : (i + 1) * 512],
                                    start=(mm == 0),
                                    stop=(mm == n_mm - 1),
                                )
                                mm += 1
                            for i in range(len(group)):
                                nc.tensor.matmul(
                                    sum_ps[:],
                                    ones_sb[:],
                                    ex[:, i * 512 : (i + 1) * 512],
                                    start=(mm - len(group) + i == 0),
                                    stop=(mm - len(group) + i == n_mm - 1),
                                )

                        pend = None
                        for group in groups:
                            gw = len(group) * 512
                            sc_ps = pS.tile([128, 1024], F32, name="sc_ps")
                            for i, kt in enumerate(group):
                                nc.tensor.matmul(
                                    sc_ps[:, i * 512 : (i + 1) * 512],
                                    kT_sb[:, h * S + kt * 128 : h * S + (kt + 1) * 128],
                                    qT_sb[:, h * S + q0 : h * S + q0 + 512],
                                    start=True,
                                    stop=True,
                                )
                            masked = any(
                                cls_grid[kt][qc] == B_ADD for kt in group
                            )
                            if masked and not causal:
                                mk = p2_m.tile([128, 1024], F32, name="mk")
                                contig = group == list(
                                    range(group[0], group[0] + len(group))
                                )
                                if contig:
                                    kt0 = group[0]
                                    nkt = len(group)
                                    nc.scalar.dma_start(
                                        mk[:, : nkt * 512].rearrange(
                                            "p (t q) -> p t q", q=512
                                        ),
                                        maskT.ap()[
                                            kt0 * 128 : (kt0 + nkt) * 128,
                                            q0 : q0 + 512,
                                        ].rearrange("(t p) q -> p t q", p=128),
                                    )
                                else:
                                    for i, kt in enumerate(group):
                                        nc.scalar.dma_start(
                                            mk[:, i * 512 : (i + 1) * 512],
                                            maskT.ap()[
                                                kt * 128 : (kt + 1) * 128,
                                                q0 : q0 + 512,
                                            ],
                                        )
                                nc.vector.tensor_add(
                                    sc_ps[:, :gw], sc_ps[:, :gw], mk[:, :gw]
                                )
                            ex = p2_ex.tile([128, 1024], BF16, name="ex")
                            last_b["scalar"] = nc.scalar.activation(
                                ex[:, :gw],
                                sc_ps[:, :gw],
                                mybir.ActivationFunctionType.Exp,
                                scale=inv_sqrt_hd,
                            )
                            if masked and causal:
                                # zero the above-diagonal region (bf16, DVE);
                                # diagonal groups sit at kt offset group[0]-4qc
                                kt_off = group[0] - 4 * qc
                                nc.vector.tensor_mul(
                                    ex[:, :gw],
                                    ex[:, :gw],
                                    stair_sb[:, kt_off * 512 : kt_off * 512 + gw],
                                )
                            if pend is not None:
                                flush(pend)
                            pend = (group, ex)
                        flush(pend)
                        rec = p2_sm.tile([1, 512], F32, name="rec")
                        nc.vector.reciprocal_approx_fast(rec[:], sum_ps[0:1, :])
                        rb = p2_sm.tile([128, 512], F32, name="rb")
                        nc.gpsimd.partition_broadcast(rb[:], rec[0:1, :])
                        at = p2_at.tile([128, 512], BF16, name="at")
                        nc.vector.tensor_mul(at[:], att_ps[:], rb[:])
                        last_b["sync"] = nc.sync.dma_start(
                            attn_sc[qc].ap()[h * 128 : (h + 1) * 128, :], at[:]
                        )
                    pend_ag.append(qc)
                    if qc >= 1:
                        ag = pend_ag.pop(0)
                        nc.gpsimd.collective_compute(
                            "AllGather",
                            mybir.AluOpType.bypass,
                            ins=[attn_sc[ag].ap()],
                            outs=[attn_full[ag].ap()],
                            replica_groups=[list(range(N_CORES))],
                        )
                if pend_ag:
                    for ag in pend_ag:
                        nc.gpsimd.collective_compute(
                            "AllGather",
                            mybir.AluOpType.bypass,
                            ins=[attn_sc[ag].ap()],
                            outs=[attn_full[ag].ap()],
                            replica_groups=[list(range(N_CORES))],
                        )

            # ---- phase C: output projection per 512-seq chunk ----
            with tc.tile_pool(name="p3_ps", bufs=1, space="PSUM") as pC:
                for qc in range(SC):
                    q0 = qc * 512
                    pso = [
                        pC.tile([128, 512], F32, name=f"pso{i}") for i in range(HL)
                    ]
                    for blk in range(4):  # 8 kc per block
                        att = []
                        for t in range(4):
                            dc2 = blk * 4 + t
                            at_t = p3_a.tile(
                                [128, 1024], BF16, name=f"at_t{t}"
                            )
                            ename = "sync" if t % 2 == 0 else "scalar"
                            ld = getattr(nc, ename).dma_start(
                                at_t[:].rearrange("p (two s) -> p two s", s=512),
                                attn_full[qc]
                                .ap()[dc2 * 256 : (dc2 + 1) * 256, :]
                                .rearrange("(two p) s -> p two s", p=128),
                            )
                            if qc == 0 and blk < 2 and ename in last_b:
                                add_dep_helper(
                                    ld.ins,
                                    last_b[ename].ins,
                                    sync=False,
                                    reason="C loads stay behind B on this queue",
                                )
                            att.append(at_t)
                        for jt in range(HL):
                            for kci in range(8):
                                kc = blk * 8 + kci
                                nc.tensor.matmul(
                                    pso[jt][:],
                                    wo_sb[:, kc * DSH + jt * 128 : kc * DSH + (jt + 1) * 128],
                                    att[kci // 2][:, (kci % 2) * 512 : (kci % 2) * 512 + 512],
                                    start=(kc == 0),
                                    stop=(kc == KT - 1),
                                )
                    for jt in range(HL):
                        oev = p3_ev.tile([128, 512], F32, name="oev")
                        nc.vector.tensor_copy(oev[:], pso[jt][:])
                        nc.sync.dma_start(
                            outT.ap()[jt * 128 : (jt + 1) * 128, q0 : q0 + 512],
                            oev[:],
                        )

    nc.compile()
    return nc


def _install_trace_hooks():
    """Install the NTFF profile hook (missing antenv.axon_hooks stub) and
    neutralize the artifact upload so trace=True works in this container."""
    import sys
    import types

    from concourse import bass_utils as _bu

    _bu.upload_artifacts = lambda tmpdir: f"file://{tmpdir}"
    if "antenv.axon_hooks" in sys.modules:
        return
    import antenv

    mod = types.ModuleType("antenv.axon_hooks")
    _h = [None]
    mod.set_axon_ntff_profile_hook = lambda hk: _h.__setitem__(0, hk)
    mod.get_axon_ntff_profile_hook = lambda: _h[0]
    sys.modules["antenv.axon_hooks"] = mod
    antenv.axon_hooks = mod
    from trn_agent_boot.trn_boot import _ntff_profile_via_ctypes

    mod.set_axon_ntff_profile_hook(
        _ntff_profile_via_ctypes("/opt/axon/libaxon_pjrt.so")
    )


_CACHE = {}


def _get_program(cls_grid, causal):
    key = (tuple(map(tuple, cls_grid)), causal)
    if key not in _CACHE:
        _CACHE[key] = _build(cls_grid, causal)
    return _CACHE[key]


def _classify_mask(maskT_np):
    """Classify each [128k, 512q] block of the transposed mask."""
    grid = []
    for kt in range(ST):
        row = []
        for qc in range(SC):
            blk = maskT_np[kt * 128 : (kt + 1) * 128, qc * 512 : (qc + 1) * 512]
            if np.all(blk < -1e4):
                row.append(B_SKIP)
            elif np.all(blk == 0.0):
                row.append(B_ZERO)
            else:
                row.append(B_ADD)
        grid.append(row)
    return grid


_ONES = np.ones((128, 1), dtype=ml_dtypes.bfloat16)
_WARM = np.zeros((128, 512), dtype=ml_dtypes.bfloat16)

# 0/1 staircase for diagonal mask groups: stair[p, ji*512 + q] = (ji*128+p <= q)
_STAIR = np.zeros((128, 2048), dtype=ml_dtypes.bfloat16)
for _ji in range(4):
    for _p in range(128):
        _q0 = _ji * 128 + _p
        if _q0 < 512:
            _STAIR[_p, _ji * 512 + _q0 : (_ji + 1) * 512] = 1.0

# within-head permutation: even head_dim indices first, then odd
_PERM = np.empty(DSH, dtype=np.int64)
for _hl in range(HL):
    for _j in range(64):
        _PERM[_hl * 128 + _j] = _hl * 128 + 2 * _j
        _PERM[_hl * 128 + 64 + _j] = _hl * 128 + 2 * _j + 1


def kernel(x, start_pos, freqs, mask, wq, wk, wv, wo):
    x = np.asarray(x, dtype=np.float32)
    freqs = np.asarray(freqs, dtype=np.float32)
    mask = np.asarray(mask, dtype=np.float32)
    wq = np.asarray(wq, dtype=np.float32)
    wk = np.asarray(wk, dtype=np.float32)
    wv = np.asarray(wv, dtype=np.float32)
    wo = np.asarray(wo, dtype=np.float32)

    xs = x.reshape(S, D)
    xT_bf = np.ascontiguousarray(xs.T).astype(ml_dtypes.bfloat16)
    # rotary multipliers, head-dim permuted: rows 0-63 cos-sin, 64-127 cos+sin
    gk_np = np.ascontiguousarray(
        np.concatenate(
            [
                (freqs[:, :, 0] - freqs[:, :, 1]).T,
                (freqs[:, :, 0] + freqs[:, :, 1]).T,
            ],
            axis=0,
        ).astype(np.float32)
    )  # [128, S]
    mask2d = mask.reshape(S, S)
    causal = bool(
        np.array_equal(
            mask2d, np.triu(np.full((S, S), -1e9, dtype=np.float32), k=1)
        )
    )
    maskT_np = np.ascontiguousarray(mask2d.T)
    cls_grid = _classify_mask(maskT_np)
    nc = _get_program(cls_grid, causal)

    in_maps = []
    for c in range(N_CORES):
        rows = slice(c * DSH, (c + 1) * DSH)
        wq_c = wq[rows][_PERM]  # permute within-head rows (even hd, odd hd)
        wk_c = wk[rows][_PERM]
        im = {
            "xT": xT_bf,
            "wqT": np.ascontiguousarray(wq_c.T).astype(ml_dtypes.bfloat16),
            "wkT": np.ascontiguousarray(wk_c.T).astype(ml_dtypes.bfloat16),
            "wvT": np.ascontiguousarray(wv[rows].T).astype(ml_dtypes.bfloat16),
            "woT": np.ascontiguousarray(wo[rows].T).astype(ml_dtypes.bfloat16),
            "gk_d": gk_np,
            "ones_in": _ONES,
            "warm_d": _WARM,
        }
        if causal:
            im["stair_d"] = _STAIR
        else:
            im["maskT"] = maskT_np
        in_maps.append(im)

    trace = os.environ.get("ATTN_TRACE") == "1"
    if trace:
        try:
            _install_trace_hooks()
        except Exception:
            pass

    res = run_bass_kernel_spmd(
        nc,
        in_maps,
        list(range(N_CORES)),
        trace=trace,
        trace_cores=[0] if trace else None,
    )
    if trace:
        kernel.last_exec_time_ns = res.exec_time_ns
        kernel.last_results = res

    out = np.empty((S, D), dtype=np.float32)
    for c in range(N_CORES):
        out[:, c * DSH : (c + 1) * DSH] = res.results[c]["outT"].T
    return out[None]
